# revision 2
# baseline (speedup 1.0000x reference)
"""Multi-head attention + out-projection on 8 TRN2 NeuronCores.

Reference computation (per batch b, head h):
    S = Q K^T / sqrt(64);  P = softmax(S, axis=-1);  O = P V
    OUT = O @ W_out^T + b_out

Sharding: B*H = 64 (b,h) pairs split across 8 cores (8 pairs/core);
attention is fully local per pair, out-proj weights replicated.

Device-side layout choices (host prep is plain numpy, free):
  - Q^T, K^T passed pre-transposed as [pairs, 64, 2048] bf16 so the
    contraction dim (d) is the partition dim for both matmul operands.
  - V passed as [pairs, 128, 16, 65] bf16, k-tiled p-major, with a
    ones-column appended; the PV matmul then yields both O^T and the
    softmax row-sums in one accumulation (out partition 64 = rowsum).
  - S^T tiles [128 k, 1024 q] in PSUM; exp on ScalarE with scale=1/8
    folded into the activation; no max-subtraction (scores are O(+-7),
    exp stays comfortably inside f32/bf16 range).
  - out-proj: lhsT = normalized O^T slices, rhs = W_out^T -> natural
    [q, e] output; bias added by VectorE from a pre-broadcast tile.
"""

import numpy as np
import ml_dtypes

from concourse import bacc, tile, mybir
from concourse.bass_utils import run_bass_kernel_spmd

B, H, S, D = 4, 16, 2048, 64
NCORES = 8
PAIRS = (B * H) // NCORES  # 8 (b,h) pairs per core
NKT = S // 128             # 16 key tiles
NQT = S // 128             # 16 query tiles
CHUNK = 1024               # query-column chunk (2 PSUM banks)
NCHUNK = S // CHUNK

_NC_CACHE = {}


def build_nc():
    f32, bf16 = mybir.dt.float32, mybir.dt.bfloat16
    nc = bacc.Bacc(None, target_bir_lowering=False)

    qt_d = nc.declare_dram_parameter("qt", [PAIRS, D, S], bf16, isOutput=False)
    kt_d = nc.declare_dram_parameter("kt", [PAIRS, D, S], bf16, isOutput=False)
    vh_d = nc.declare_dram_parameter("vh", [PAIRS, 128, NKT, D + 1], bf16, isOutput=False)
    wt_d = nc.declare_dram_parameter("wt", [D, D], bf16, isOutput=False)
    bb_d = nc.declare_dram_parameter("bb", [128, D], f32, isOutput=False)
    out_d = nc.declare_dram_parameter("out", [PAIRS, 128, NQT, D], f32, isOutput=True)

    EXPF = mybir.ActivationFunctionType.Exp
    MULT = mybir.AluOpType.mult
    ADD = mybir.AluOpType.add

    with tile.TileContext(nc) as tc:
        with (
            tc.tile_pool(name="const", bufs=1) as constp,
            tc.tile_pool(name="qk", bufs=2) as qkp,
            tc.tile_pool(name="vhp", bufs=2) as vhp,
            tc.tile_pool(name="pt", bufs=4) as ptp,
            tc.tile_pool(name="ep", bufs=2) as epp,
            tc.tile_pool(name="osb", bufs=2) as osbp,
            tc.tile_pool(name="spsum", bufs=2, space="PSUM") as spsum,
            tc.tile_pool(name="opsum", bufs=1, space="PSUM") as opsum,
            tc.tile_pool(name="prpsum", bufs=2, space="PSUM") as prpsum,
        ):
            wt_sb = constp.tile([D, D], bf16)
            nc.sync.dma_start(wt_sb[:], wt_d[:])
            bb_sb = constp.tile([128, D], f32)
            nc.sync.dma_start(bb_sb[:], bb_d[:])
            zb = constp.tile([128, 1], f32)
            nc.vector.memset(zb[:], 0.0)

            for pp in range(PAIRS):
                qt_sb = qkp.tile([D, S], bf16, tag="qt")
                nc.sync.dma_start(qt_sb[:], qt_d[pp])
                kt_sb = qkp.tile([D, S], bf16, tag="kt")
                nc.sync.dma_start(kt_sb[:], kt_d[pp])
                vh_sb = vhp.tile([128, NKT, D + 1], bf16)
                nc.sync.dma_start(vh_sb[:], vh_d[pp])
                out_sb = osbp.tile([128, NQT, D], f32)

                for c in range(NCHUNK):
                    q0 = c * CHUNK
                    o_ps = opsum.tile([D + 1, CHUNK], f32)
                    for k in range(NKT):
                        s_ps = spsum.tile([128, CHUNK], f32)
                        for j in (0, 1):
                            nc.tensor.matmul(
                                s_ps[:, j * 512:(j + 1) * 512],
                                kt_sb[:, k * 128:(k + 1) * 128],
                                qt_sb[:, q0 + j * 512:q0 + (j + 1) * 512],
                                start=True, stop=True,
                            )
                        p_sb = ptp.tile([128, CHUNK], bf16)
                        nc.scalar.activation(p_sb[:], s_ps[:], EXPF, bias=zb[:], scale=0.125)
                        for j in (0, 1):
                            nc.tensor.matmul(
                                o_ps[:, j * 512:(j + 1) * 512],
                                vh_sb[:, k, :],
                                p_sb[:, j * 512:(j + 1) * 512],
                                start=(k == 0), stop=(k == NKT - 1),
                            )

                    rs = epp.tile([1, CHUNK], f32, tag="rs")
                    nc.vector.reciprocal(rs[:], o_ps[D:D + 1, :])
                    rb = epp.tile([D, CHUNK], f32, tag="rb")
                    nc.gpsimd.partition_broadcast(rb[:], rs[:])
                    on_sb = epp.tile([D, CHUNK], bf16, tag="on")
                    nc.vector.tensor_tensor(on_sb[:], o_ps[0:D, :], rb[:], MULT)

                    for t in range(CHUNK // 128):
                        tt = (CHUNK // 128) * c + t
                        pr_ps = prpsum.tile([128, D], f32)
                        nc.tensor.matmul(
                            pr_ps[:],
                            on_sb[:, t * 128:(t + 1) * 128],
                            wt_sb[:],
                            start=True, stop=True,
                        )
                        nc.vector.tensor_tensor(out_sb[:, tt, :], pr_ps[:], bb_sb[:], ADD)

                nc.sync.dma_start(out_d[pp], out_sb[:])

    nc.compile()
    return nc


def kernel(queries, keys, values, W_out, b_out):
    bf16 = ml_dtypes.bfloat16

    q = np.asarray(queries, dtype=np.float32).reshape(B * H, S, D)
    k = np.asarray(keys, dtype=np.float32).reshape(B * H, S, D)
    v = np.asarray(values, dtype=np.float32).reshape(B * H, S, D)

    wt = np.ascontiguousarray(np.asarray(W_out, dtype=np.float32).T).astype(bf16)
    bb = np.ascontiguousarray(
        np.broadcast_to(np.asarray(b_out, dtype=np.float32), (128, D))
    )

    in_maps = []
    for c in range(NCORES):
        sl = slice(c * PAIRS, (c + 1) * PAIRS)
        qt = np.ascontiguousarray(q[sl].transpose(0, 2, 1)).astype(bf16)
        kt = np.ascontiguousarray(k[sl].transpose(0, 2, 1)).astype(bf16)
        # [pairs, S, D] -> k-tiled p-major [pairs, 128, NKT, D], append ones col
        vt = v[sl].reshape(PAIRS, NKT, 128, D).transpose(0, 2, 1, 3)
        vh = np.empty((PAIRS, 128, NKT, D + 1), dtype=bf16)
        vh[..., :D] = vt.astype(bf16)
        vh[..., D] = 1.0
        in_maps.append({"qt": qt, "kt": kt, "vh": vh, "wt": wt, "bb": bb})

    if "nc" not in _NC_CACHE:
        _NC_CACHE["nc"] = build_nc()
    nc = _NC_CACHE["nc"]

    global _LAST_IN_MAPS
    _LAST_IN_MAPS = in_maps

    res = run_bass_kernel_spmd(nc, in_maps, list(range(NCORES)))

    out = np.empty((B * H, S, D), dtype=np.float32)
    for c in range(NCORES):
        o = res.results[c]["out"]  # [PAIRS, 128, NQT, D], q = t*128 + p
        out[c * PAIRS:(c + 1) * PAIRS] = (
            o.transpose(0, 2, 1, 3).reshape(PAIRS, S, D)
        )
    return out.reshape(B, H, S, D)


# revision 6
# speedup vs baseline: 1.3038x; 1.3038x over previous
"""Multi-head attention + out-projection on 8 TRN2 NeuronCores.

Reference computation (per batch b, head h):
    S = Q K^T / sqrt(64);  P = softmax(S, axis=-1);  O = P V
    OUT = O @ W_out^T + b_out

Sharding: B*H = 64 (b,h) pairs split across 8 cores (8 pairs/core);
attention is fully local per pair, out-proj weights replicated.

Device-side structure (per core):
  - Pairs are processed two at a time (A,B) stacked in SBUF partitions
    0-63 / 64-127. The QK^T matmuls contract over d=64, so A and B
    land on disjoint PE row-groups (tile_position (0,0) vs (64,0)) and
    run concurrently on the systolic array; the per-head S^T PSUM
    tiles also act as each other's double buffer.
  - S^T tiles [128 k, 1024 q] in PSUM; exp on ScalarE with the 1/8
    score scale folded into the activation; no max-subtraction
    (scores are O(+-7), exp stays inside f32/bf16 range).
  - V carries an appended ones-column so the PV matmul produces both
    O^T (partitions 0-63) and the softmax row-sums (partition 64) in
    one accumulation.
  - Normalization: copy O^T to SBUF, DMA-broadcast the rowsum row
    across partitions (stride-0 source AP), reciprocal + multiply on
    VectorE.
  - Out-proj: lhsT = normalized O^T slices, rhs = W_out^T -> natural
    [q, e] layout; the 8 q-tiles of a chunk share one PSUM bank; bias
    added by VectorE in one [128, 512] op per chunk.

Host prep (plain numpy, free): Q/K pre-transposed to [d, s] bf16;
V k-tiled p-major with ones-column, bf16; W_out^T bf16; bias
pre-broadcast/tiled f32.
"""

import numpy as np
import ml_dtypes

from concourse import bass, bacc, tile, mybir
from concourse.bass_utils import run_bass_kernel_spmd

B, H, S, D = 4, 16, 2048, 64
NCORES = 8
PAIRS = (B * H) // NCORES  # 8 (b,h) pairs per core
NKT = S // 128             # 16 key tiles
NQT = S // 128             # 16 query tiles
CHUNK = 1024               # query-column chunk (2 PSUM banks)
NCHUNK = S // CHUNK

_NC_CACHE = {}


def build_nc():
    f32, bf16 = mybir.dt.float32, mybir.dt.bfloat16
    nc = bacc.Bacc(None, target_bir_lowering=False)

    qt_d = nc.declare_dram_parameter("qt", [PAIRS, D, S], bf16, isOutput=False)
    kt_d = nc.declare_dram_parameter("kt", [PAIRS, D, S], bf16, isOutput=False)
    vh_d = nc.declare_dram_parameter("vh", [PAIRS, 128, NKT, D + 1], bf16, isOutput=False)
    wt_d = nc.declare_dram_parameter("wt", [D, D], bf16, isOutput=False)
    bb_d = nc.declare_dram_parameter("bb", [128, 8 * D], f32, isOutput=False)
    out_d = nc.declare_dram_parameter("out", [PAIRS, 128, NQT * D], f32, isOutput=True)

    EXPF = mybir.ActivationFunctionType.Exp
    MULT = mybir.AluOpType.mult
    ADD = mybir.AluOpType.add

    with tile.TileContext(nc) as tc:
        with (
            tc.tile_pool(name="const", bufs=1) as constp,
            tc.tile_pool(name="qk", bufs=2) as qkp,
            tc.tile_pool(name="vhp", bufs=2) as vhp,
            tc.tile_pool(name="pt", bufs=6) as ptp,
            tc.tile_pool(name="ep", bufs=2) as epp,
            tc.tile_pool(name="osb", bufs=2) as osbp,
            tc.tile_pool(name="sApsum", bufs=1, space="PSUM") as sAp,
            tc.tile_pool(name="sBpsum", bufs=1, space="PSUM") as sBp,
            tc.tile_pool(name="opsum", bufs=2, space="PSUM") as opsum,
        ):
            wt_sb = constp.tile([D, D], bf16)
            nc.sync.dma_start(wt_sb[:], wt_d[:])
            bb_sb = constp.tile([128, 8 * D], f32)
            nc.sync.dma_start(bb_sb[:], bb_d[:])
            zb = constp.tile([128, 1], f32)
            nc.vector.memset(zb[:], 0.0)

            for pq in range(PAIRS // 2):
                pa, pb = 2 * pq, 2 * pq + 1
                qt2 = qkp.tile([128, S], bf16, tag="qt")
                nc.sync.dma_start(qt2[0:D, :], qt_d[pa])
                nc.sync.dma_start(qt2[D:128, :], qt_d[pb])
                kt2 = qkp.tile([128, S], bf16, tag="kt")
                nc.sync.dma_start(kt2[0:D, :], kt_d[pa])
                nc.sync.dma_start(kt2[D:128, :], kt_d[pb])
                vh2 = vhp.tile([128, 2, NKT, D + 1], bf16)
                nc.sync.dma_start(vh2[:, 0, :, :], vh_d[pa])
                nc.sync.dma_start(vh2[:, 1, :, :], vh_d[pb])
                out2 = [
                    osbp.tile([128, NQT * D], f32, tag="outA", name=f"out_{pq}_A"),
                    osbp.tile([128, NQT * D], f32, tag="outB", name=f"out_{pq}_B"),
                ]

                for c in range(NCHUNK):
                    q0 = c * CHUNK
                    o_ps = [
                        opsum.tile([D + 1, CHUNK], f32, tag="o", name=f"oA_{pq}_{c}"),
                        opsum.tile([D + 1, CHUNK], f32, tag="o", name=f"oB_{pq}_{c}"),
                    ]
                    for k in range(NKT):
                        s_ps = [
                            sAp.tile([128, CHUNK], f32, tag="s", name=f"sA_{pq}_{c}_{k}"),
                            sBp.tile([128, CHUNK], f32, tag="s", name=f"sB_{pq}_{c}_{k}"),
                        ]
                        # A/B matmul pairs adjacent in program order ->
                        # disjoint PE row-groups run concurrently.
                        for j in (0, 1):
                            for x in (0, 1):
                                nc.tensor.matmul(
                                    s_ps[x][:, j * 512:(j + 1) * 512],
                                    kt2[x * D:(x + 1) * D, k * 128:(k + 1) * 128],
                                    qt2[x * D:(x + 1) * D, q0 + j * 512:q0 + (j + 1) * 512],
                                    start=True, stop=True,
                                )
                        p_sb = [None, None]
                        for x in (0, 1):
                            p_sb[x] = ptp.tile([128, CHUNK], bf16, tag="p", name=f"p_{pq}_{c}_{k}_{x}")
                            nc.scalar.activation(p_sb[x][:], s_ps[x][:], EXPF, bias=zb[:], scale=0.125)
                        for x in (0, 1):
                            for j in (0, 1):
                                nc.tensor.matmul(
                                    o_ps[x][:, j * 512:(j + 1) * 512],
                                    vh2[:, x, k, :],
                                    p_sb[x][:, j * 512:(j + 1) * 512],
                                    start=(k == 0), stop=(k == NKT - 1),
                                )

                    for x in (0, 1):
                        o_sb = epp.tile([D + 1, CHUNK], f32, tag="osb", name=f"osb_{pq}_{c}_{x}")
                        nc.vector.tensor_copy(o_sb[:], o_ps[x][:])
                        rs = epp.tile([1, CHUNK], f32, tag="rs", name=f"rs_{pq}_{c}_{x}")
                        nc.sync.dma_start(rs[:], o_sb[D:D + 1, :])
                        rb = epp.tile([D, CHUNK], f32, tag="rb", name=f"rb_{pq}_{c}_{x}")
                        nc.gpsimd.partition_broadcast(rb[:], rs[:])
                        nc.vector.reciprocal(rb[:], rb[:])
                        on_sb = epp.tile([D, CHUNK], bf16, tag="on", name=f"on_{pq}_{c}_{x}")
                        nc.vector.tensor_tensor(on_sb[:], o_sb[0:D, :], rb[:], MULT)

                        opj = opsum.tile([128, 8 * D], f32, tag="o", name=f"opj_{pq}_{c}_{x}")
                        for t in range(CHUNK // 128):
                            nc.tensor.matmul(
                                opj[:, t * D:(t + 1) * D],
                                on_sb[:, t * 128:(t + 1) * 128],
                                wt_sb[:],
                                start=True, stop=True,
                            )
                        nc.vector.tensor_tensor(
                            out2[x][:, c * 8 * D:(c + 1) * 8 * D], opj[:], bb_sb[:], ADD
                        )

                nc.sync.dma_start(out_d[pa], out2[0][:])
                nc.sync.dma_start(out_d[pb], out2[1][:])

    nc.compile()
    return nc


def kernel(queries, keys, values, W_out, b_out):
    bf16 = ml_dtypes.bfloat16

    q = np.asarray(queries, dtype=np.float32).reshape(B * H, S, D)
    k = np.asarray(keys, dtype=np.float32).reshape(B * H, S, D)
    v = np.asarray(values, dtype=np.float32).reshape(B * H, S, D)

    wt = np.ascontiguousarray(np.asarray(W_out, dtype=np.float32).T).astype(bf16)
    bb = np.ascontiguousarray(
        np.tile(np.asarray(b_out, dtype=np.float32), (128, 8))
    )

    in_maps = []
    for c in range(NCORES):
        sl = slice(c * PAIRS, (c + 1) * PAIRS)
        qt = np.ascontiguousarray(q[sl].transpose(0, 2, 1)).astype(bf16)
        kt = np.ascontiguousarray(k[sl].transpose(0, 2, 1)).astype(bf16)
        # [pairs, S, D] -> k-tiled p-major [pairs, 128, NKT, D], append ones col
        vt = v[sl].reshape(PAIRS, NKT, 128, D).transpose(0, 2, 1, 3)
        vh = np.empty((PAIRS, 128, NKT, D + 1), dtype=bf16)
        vh[..., :D] = vt.astype(bf16)
        vh[..., D] = 1.0
        in_maps.append({"qt": qt, "kt": kt, "vh": vh, "wt": wt, "bb": bb})

    if "nc" not in _NC_CACHE:
        _NC_CACHE["nc"] = build_nc()
    nc = _NC_CACHE["nc"]

    global _LAST_IN_MAPS
    _LAST_IN_MAPS = in_maps

    res = run_bass_kernel_spmd(nc, in_maps, list(range(NCORES)))

    out = np.empty((B * H, S, D), dtype=np.float32)
    for c in range(NCORES):
        o = res.results[c]["out"]  # [PAIRS, 128, NQT*D], q = t*128 + p
        out[c * PAIRS:(c + 1) * PAIRS] = (
            o.reshape(PAIRS, 128, NQT, D).transpose(0, 2, 1, 3).reshape(PAIRS, S, D)
        )
    return out.reshape(B, H, S, D)


# revision 7
# speedup vs baseline: 1.5610x; 1.1973x over previous
"""Multi-head attention + out-projection on 8 TRN2 NeuronCores.

Reference computation (per batch b, head h):
    S = Q K^T / sqrt(64);  P = softmax(S, axis=-1);  O = P V
    OUT = O @ W_out^T + b_out

Sharding: B*H = 64 (b,h) pairs split across 8 cores (8 pairs/core);
attention is fully local per pair, out-proj weights replicated.

Device-side structure (per core):
  - Pairs are processed two at a time (A,B) stacked in SBUF partitions
    0-63 / 64-127. The QK^T matmuls contract over d=64, so A and B
    land on disjoint PE row-groups (tile_position (0,0) vs (64,0)) and
    run concurrently on the systolic array; the per-head S^T PSUM
    tiles also act as each other's double buffer.
  - S^T tiles [128 k, 1024 q] in PSUM; exp on ScalarE with the 1/8
    score scale folded into the activation; no max-subtraction
    (scores are O(+-7), exp stays inside f32/bf16 range).
  - V carries an appended ones-column so the PV matmul produces both
    O^T (partitions 0-63) and the softmax row-sums (partition 64) in
    one accumulation.
  - Normalization: copy O^T to SBUF, DMA-broadcast the rowsum row
    across partitions (stride-0 source AP), reciprocal + multiply on
    VectorE.
  - Out-proj: lhsT = normalized O^T slices, rhs = W_out^T -> natural
    [q, e] layout; the 8 q-tiles of a chunk share one PSUM bank; bias
    added by VectorE in one [128, 512] op per chunk.

Host prep (plain numpy, free): Q/K pre-transposed to [d, s] bf16;
V k-tiled p-major with ones-column, bf16; W_out^T bf16; bias
pre-broadcast/tiled f32.
"""

import numpy as np
import ml_dtypes

from concourse import bass, bacc, tile, mybir
from concourse.bass_utils import run_bass_kernel_spmd

B, H, S, D = 4, 16, 2048, 64
NCORES = 8
PAIRS = (B * H) // NCORES  # 8 (b,h) pairs per core
NKT = S // 128             # 16 key tiles
NQT = S // 128             # 16 query tiles
CHUNK = 1024               # query-column chunk (2 PSUM banks)
NCHUNK = S // CHUNK

_NC_CACHE = {}


def build_nc():
    f32, bf16 = mybir.dt.float32, mybir.dt.bfloat16
    nc = bacc.Bacc(None, target_bir_lowering=False)

    qt_d = nc.declare_dram_parameter("qt", [PAIRS, D, S], bf16, isOutput=False)
    kt_d = nc.declare_dram_parameter("kt", [PAIRS, D, S], bf16, isOutput=False)
    vh_d = nc.declare_dram_parameter("vh", [PAIRS, 128, NKT, D + 1], bf16, isOutput=False)
    wt_d = nc.declare_dram_parameter("wt", [D, D], bf16, isOutput=False)
    bb_d = nc.declare_dram_parameter("bb", [128, NQT * D], f32, isOutput=False)
    out_d = nc.declare_dram_parameter("out", [PAIRS, 128, NQT * D], f32, isOutput=True)

    EXPF = mybir.ActivationFunctionType.Exp
    MULT = mybir.AluOpType.mult
    ADD = mybir.AluOpType.add

    with tile.TileContext(nc) as tc:
        with (
            tc.tile_pool(name="const", bufs=1) as constp,
            tc.tile_pool(name="qk", bufs=2) as qkp,
            tc.tile_pool(name="vhp", bufs=2) as vhp,
            tc.tile_pool(name="pt", bufs=6) as ptp,
            tc.tile_pool(name="ep", bufs=2) as epp,
            tc.tile_pool(name="osb", bufs=2) as osbp,
            tc.tile_pool(name="sApsum", bufs=1, space="PSUM") as sAp,
            tc.tile_pool(name="sBpsum", bufs=1, space="PSUM") as sBp,
            tc.tile_pool(name="opsum", bufs=2, space="PSUM") as opsum,
        ):
            wt_sb = constp.tile([D, D], bf16)
            nc.sync.dma_start(wt_sb[:], wt_d[:])
            bb_sb = constp.tile([128, NQT * D], f32)
            nc.sync.dma_start(bb_sb[:], bb_d[:])
            zb = constp.tile([128, 1], f32)
            nc.vector.memset(zb[:], 0.0)

            for pq in range(PAIRS // 2):
                pa, pb = 2 * pq, 2 * pq + 1
                qt2 = qkp.tile([128, S], bf16, tag="qt")
                nc.sync.dma_start(qt2[0:D, :], qt_d[pa])
                nc.sync.dma_start(qt2[D:128, :], qt_d[pb])
                kt2 = qkp.tile([128, S], bf16, tag="kt")
                nc.sync.dma_start(kt2[0:D, :], kt_d[pa])
                nc.sync.dma_start(kt2[D:128, :], kt_d[pb])
                vh2 = vhp.tile([128, 2, NKT, D + 1], bf16)
                nc.sync.dma_start(vh2[:, 0, :, :], vh_d[pa])
                nc.sync.dma_start(vh2[:, 1, :, :], vh_d[pb])
                out2 = [
                    osbp.tile([128, NQT * D], f32, tag="outA", name=f"out_{pq}_A"),
                    osbp.tile([128, NQT * D], f32, tag="outB", name=f"out_{pq}_B"),
                ]
                on2 = [
                    epp.tile([D, S], bf16, tag="onA", name=f"on_{pq}_A"),
                    epp.tile([D, S], bf16, tag="onB", name=f"on_{pq}_B"),
                ]

                for c in range(NCHUNK):
                    q0 = c * CHUNK
                    o_ps = [
                        opsum.tile([D + 1, CHUNK], f32, tag="o", name=f"oA_{pq}_{c}"),
                        opsum.tile([D + 1, CHUNK], f32, tag="o", name=f"oB_{pq}_{c}"),
                    ]
                    for k in range(NKT):
                        s_ps = [
                            sAp.tile([128, CHUNK], f32, tag="s", name=f"sA_{pq}_{c}_{k}"),
                            sBp.tile([128, CHUNK], f32, tag="s", name=f"sB_{pq}_{c}_{k}"),
                        ]
                        # A/B matmul pairs adjacent in program order ->
                        # disjoint PE row-groups run concurrently.
                        for j in (0, 1):
                            for x in (0, 1):
                                nc.tensor.matmul(
                                    s_ps[x][:, j * 512:(j + 1) * 512],
                                    kt2[x * D:(x + 1) * D, k * 128:(k + 1) * 128],
                                    qt2[x * D:(x + 1) * D, q0 + j * 512:q0 + (j + 1) * 512],
                                    start=True, stop=True,
                                )
                        p_sb = [None, None]
                        for x in (0, 1):
                            p_sb[x] = ptp.tile([128, CHUNK], bf16, tag="p", name=f"p_{pq}_{c}_{k}_{x}")
                            nc.scalar.activation(p_sb[x][:], s_ps[x][:], EXPF, bias=zb[:], scale=0.125)
                        for x in (0, 1):
                            for j in (0, 1):
                                nc.tensor.matmul(
                                    o_ps[x][:, j * 512:(j + 1) * 512],
                                    vh2[:, x, k, :],
                                    p_sb[x][:, j * 512:(j + 1) * 512],
                                    start=(k == 0), stop=(k == NKT - 1),
                                )

                    for x in (0, 1):
                        o_sb = epp.tile([D + 1, CHUNK], f32, tag="osb", name=f"osb_{pq}_{c}_{x}")
                        nc.vector.tensor_copy(o_sb[:], o_ps[x][:])
                        rs = epp.tile([1, CHUNK], f32, tag="rs", name=f"rs_{pq}_{c}_{x}")
                        nc.sync.dma_start(rs[:], o_sb[D:D + 1, :])
                        rb = epp.tile([D, CHUNK], f32, tag="rb", name=f"rb_{pq}_{c}_{x}")
                        nc.gpsimd.partition_broadcast(rb[:], rs[:])
                        nc.vector.reciprocal_approx_fast(rb[:], rb[:])
                        nc.vector.tensor_tensor(
                            on2[x][:, q0:q0 + CHUNK], o_sb[0:D, :], rb[:], MULT
                        )

                # out-projection + bias once per pair, after both chunks
                for x in (0, 1):
                    opj = opsum.tile([128, NQT * D], f32, tag="o", name=f"opj_{pq}_{x}")
                    for t in range(NQT):
                        nc.tensor.matmul(
                            opj[:, t * D:(t + 1) * D],
                            on2[x][:, t * 128:(t + 1) * 128],
                            wt_sb[:],
                            start=True, stop=True,
                        )
                    nc.vector.tensor_tensor(out2[x][:], opj[:], bb_sb[:], ADD)

                nc.sync.dma_start(out_d[pa], out2[0][:])
                nc.sync.dma_start(out_d[pb], out2[1][:])

    nc.compile()
    return nc


def kernel(queries, keys, values, W_out, b_out):
    bf16 = ml_dtypes.bfloat16

    q = np.asarray(queries, dtype=np.float32).reshape(B * H, S, D)
    k = np.asarray(keys, dtype=np.float32).reshape(B * H, S, D)
    v = np.asarray(values, dtype=np.float32).reshape(B * H, S, D)

    wt = np.ascontiguousarray(np.asarray(W_out, dtype=np.float32).T).astype(bf16)
    bb = np.ascontiguousarray(
        np.tile(np.asarray(b_out, dtype=np.float32), (128, NQT))
    )

    in_maps = []
    for c in range(NCORES):
        sl = slice(c * PAIRS, (c + 1) * PAIRS)
        qt = np.ascontiguousarray(q[sl].transpose(0, 2, 1)).astype(bf16)
        kt = np.ascontiguousarray(k[sl].transpose(0, 2, 1)).astype(bf16)
        # [pairs, S, D] -> k-tiled p-major [pairs, 128, NKT, D], append ones col
        vt = v[sl].reshape(PAIRS, NKT, 128, D).transpose(0, 2, 1, 3)
        vh = np.empty((PAIRS, 128, NKT, D + 1), dtype=bf16)
        vh[..., :D] = vt.astype(bf16)
        vh[..., D] = 1.0
        in_maps.append({"qt": qt, "kt": kt, "vh": vh, "wt": wt, "bb": bb})

    if "nc" not in _NC_CACHE:
        _NC_CACHE["nc"] = build_nc()
    nc = _NC_CACHE["nc"]

    global _LAST_IN_MAPS
    _LAST_IN_MAPS = in_maps

    res = run_bass_kernel_spmd(nc, in_maps, list(range(NCORES)))

    out = np.empty((B * H, S, D), dtype=np.float32)
    for c in range(NCORES):
        o = res.results[c]["out"]  # [PAIRS, 128, NQT*D], q = t*128 + p
        out[c * PAIRS:(c + 1) * PAIRS] = (
            o.reshape(PAIRS, 128, NQT, D).transpose(0, 2, 1, 3).reshape(PAIRS, S, D)
        )
    return out.reshape(B, H, S, D)


# revision 8
# speedup vs baseline: 1.5925x; 1.0202x over previous
"""Multi-head attention + out-projection on 8 TRN2 NeuronCores.

Reference computation (per batch b, head h):
    S = Q K^T / sqrt(64);  P = softmax(S, axis=-1);  O = P V
    OUT = O @ W_out^T + b_out

Sharding: B*H = 64 (b,h) pairs split across 8 cores (8 pairs/core);
attention is fully local per pair, out-proj weights replicated.

Device-side structure (per core):
  - Pairs are processed two at a time (A,B) stacked in SBUF partitions
    0-63 / 64-127. The QK^T matmuls contract over d=64, so A and B
    land on disjoint PE row-groups (tile_position (0,0) vs (64,0)) and
    run concurrently on the systolic array; the per-head S^T PSUM
    tiles also act as each other's double buffer.
  - S^T tiles [128 k, 1024 q] in PSUM; exp on ScalarE with the 1/8
    score scale folded into the activation; no max-subtraction
    (scores are O(+-7), exp stays inside f32/bf16 range).
  - V carries an appended ones-column so the PV matmul produces both
    O^T (partitions 0-63) and the softmax row-sums (partition 64) in
    one accumulation.
  - Normalization: copy O^T to SBUF, DMA-broadcast the rowsum row
    across partitions (stride-0 source AP), reciprocal + multiply on
    VectorE.
  - Out-proj: lhsT = normalized O^T slices, rhs = W_out^T -> natural
    [q, e] layout; the 8 q-tiles of a chunk share one PSUM bank; bias
    added by VectorE in one [128, 512] op per chunk.

Host prep (plain numpy, free): Q/K pre-transposed to [d, s] bf16;
V k-tiled p-major with ones-column, bf16; W_out^T bf16; bias
pre-broadcast/tiled f32.
"""

import numpy as np
import ml_dtypes

from concourse import bass, bacc, tile, mybir
from concourse.bass_utils import run_bass_kernel_spmd

B, H, S, D = 4, 16, 2048, 64
NCORES = 8
PAIRS = (B * H) // NCORES  # 8 (b,h) pairs per core
NKT = S // 128             # 16 key tiles
NQT = S // 128             # 16 query tiles
CHUNK = 1024               # query-column chunk (2 PSUM banks)
NCHUNK = S // CHUNK

_NC_CACHE = {}


def build_nc():
    f32, bf16 = mybir.dt.float32, mybir.dt.bfloat16
    nc = bacc.Bacc(None, target_bir_lowering=False)

    qt_d = nc.declare_dram_parameter("qt", [PAIRS, D, S], bf16, isOutput=False)
    kt_d = nc.declare_dram_parameter("kt", [PAIRS, D, S], bf16, isOutput=False)
    vh_d = nc.declare_dram_parameter("vh", [PAIRS, 128, NKT, D + 1], bf16, isOutput=False)
    wt_d = nc.declare_dram_parameter("wt", [D, D], bf16, isOutput=False)
    bb_d = nc.declare_dram_parameter("bb", [128, NQT * D], f32, isOutput=False)
    out_d = nc.declare_dram_parameter("out", [PAIRS, 128, NQT * D], f32, isOutput=True)

    EXPF = mybir.ActivationFunctionType.Exp
    MULT = mybir.AluOpType.mult
    ADD = mybir.AluOpType.add

    with tile.TileContext(nc) as tc:
        with (
            tc.tile_pool(name="const", bufs=1) as constp,
            tc.tile_pool(name="qk", bufs=2) as qkp,
            tc.tile_pool(name="vhp", bufs=2) as vhp,
            tc.tile_pool(name="pt", bufs=8) as ptp,
            tc.tile_pool(name="ep", bufs=2) as epp,
            tc.tile_pool(name="osb", bufs=2) as osbp,
            tc.tile_pool(name="sApsum", bufs=1, space="PSUM") as sAp,
            tc.tile_pool(name="sBpsum", bufs=1, space="PSUM") as sBp,
            tc.tile_pool(name="opsum", bufs=2, space="PSUM") as opsum,
        ):
            wt_sb = constp.tile([D, D], bf16)
            nc.sync.dma_start(wt_sb[:], wt_d[:])
            bb_sb = constp.tile([128, NQT * D], f32)
            nc.sync.dma_start(bb_sb[:], bb_d[:])
            zb = constp.tile([128, 1], f32)
            nc.vector.memset(zb[:], 0.0)

            for pq in range(PAIRS // 2):
                pa, pb = 2 * pq, 2 * pq + 1
                qt2 = qkp.tile([128, S], bf16, tag="qt")
                nc.sync.dma_start(qt2[0:D, :], qt_d[pa])
                nc.sync.dma_start(qt2[D:128, :], qt_d[pb])
                kt2 = qkp.tile([128, S], bf16, tag="kt")
                nc.sync.dma_start(kt2[0:D, :], kt_d[pa])
                nc.sync.dma_start(kt2[D:128, :], kt_d[pb])
                vh2 = vhp.tile([128, 2, NKT, D + 1], bf16)
                nc.sync.dma_start(vh2[:, 0, :, :], vh_d[pa])
                nc.sync.dma_start(vh2[:, 1, :, :], vh_d[pb])
                out2 = [
                    osbp.tile([128, NQT * D], f32, tag="outA", name=f"out_{pq}_A"),
                    osbp.tile([128, NQT * D], f32, tag="outB", name=f"out_{pq}_B"),
                ]
                on2 = [
                    epp.tile([D, S], bf16, tag="onA", name=f"on_{pq}_A"),
                    epp.tile([D, S], bf16, tag="onB", name=f"on_{pq}_B"),
                ]

                for c in range(NCHUNK):
                    q0 = c * CHUNK
                    o_ps = [
                        opsum.tile([D + 1, CHUNK], f32, tag="o", name=f"oA_{pq}_{c}"),
                        opsum.tile([D + 1, CHUNK], f32, tag="o", name=f"oB_{pq}_{c}"),
                    ]
                    for k in range(NKT):
                        s_ps = [
                            sAp.tile([128, CHUNK], f32, tag="s", name=f"sA_{pq}_{c}_{k}"),
                            sBp.tile([128, CHUNK], f32, tag="s", name=f"sB_{pq}_{c}_{k}"),
                        ]
                        # Per head j=0/j=1 share the stationary operand;
                        # alternating heads every 2 MMs keeps row-groups
                        # disjoint so the next head's LDWEIGHTS pulls ahead.
                        for x in (0, 1):
                            for j in (0, 1):
                                nc.tensor.matmul(
                                    s_ps[x][:, j * 512:(j + 1) * 512],
                                    kt2[x * D:(x + 1) * D, k * 128:(k + 1) * 128],
                                    qt2[x * D:(x + 1) * D, q0 + j * 512:q0 + (j + 1) * 512],
                                    start=True, stop=True,
                                )
                        p_sb = [None, None]
                        for x in (0, 1):
                            p_sb[x] = ptp.tile([128, CHUNK], bf16, tag="p", name=f"p_{pq}_{c}_{k}_{x}")
                            nc.scalar.activation(p_sb[x][:], s_ps[x][:], EXPF, bias=zb[:], scale=0.125)
                        for x in (0, 1):
                            for j in (0, 1):
                                nc.tensor.matmul(
                                    o_ps[x][:, j * 512:(j + 1) * 512],
                                    vh2[:, x, k, :],
                                    p_sb[x][:, j * 512:(j + 1) * 512],
                                    start=(k == 0), stop=(k == NKT - 1),
                                )

                    for x in (0, 1):
                        o_sb = epp.tile([D + 1, CHUNK], f32, tag="osb", name=f"osb_{pq}_{c}_{x}")
                        nc.vector.tensor_copy(o_sb[:], o_ps[x][:])
                        rs = epp.tile([1, CHUNK], f32, tag="rs", name=f"rs_{pq}_{c}_{x}")
                        nc.sync.dma_start(rs[:], o_sb[D:D + 1, :])
                        rb = epp.tile([D, CHUNK], f32, tag="rb", name=f"rb_{pq}_{c}_{x}")
                        nc.gpsimd.partition_broadcast(rb[:], rs[:])
                        nc.vector.reciprocal_approx_fast(rb[:], rb[:])
                        nc.vector.tensor_tensor(
                            on2[x][:, q0:q0 + CHUNK], o_sb[0:D, :], rb[:], MULT
                        )

                # out-projection + bias once per pair, after both chunks
                for x in (0, 1):
                    opj = opsum.tile([128, NQT * D], f32, tag="o", name=f"opj_{pq}_{x}")
                    for t in range(NQT):
                        nc.tensor.matmul(
                            opj[:, t * D:(t + 1) * D],
                            on2[x][:, t * 128:(t + 1) * 128],
                            wt_sb[:],
                            start=True, stop=True,
                        )
                    nc.vector.tensor_tensor(out2[x][:], opj[:], bb_sb[:], ADD)

                nc.sync.dma_start(out_d[pa], out2[0][:])
                nc.sync.dma_start(out_d[pb], out2[1][:])

    nc.compile()
    return nc


def kernel(queries, keys, values, W_out, b_out):
    bf16 = ml_dtypes.bfloat16

    q = np.asarray(queries, dtype=np.float32).reshape(B * H, S, D)
    k = np.asarray(keys, dtype=np.float32).reshape(B * H, S, D)
    v = np.asarray(values, dtype=np.float32).reshape(B * H, S, D)

    wt = np.ascontiguousarray(np.asarray(W_out, dtype=np.float32).T).astype(bf16)
    bb = np.ascontiguousarray(
        np.tile(np.asarray(b_out, dtype=np.float32), (128, NQT))
    )

    in_maps = []
    for c in range(NCORES):
        sl = slice(c * PAIRS, (c + 1) * PAIRS)
        qt = np.ascontiguousarray(q[sl].transpose(0, 2, 1)).astype(bf16)
        kt = np.ascontiguousarray(k[sl].transpose(0, 2, 1)).astype(bf16)
        # [pairs, S, D] -> k-tiled p-major [pairs, 128, NKT, D], append ones col
        vt = v[sl].reshape(PAIRS, NKT, 128, D).transpose(0, 2, 1, 3)
        vh = np.empty((PAIRS, 128, NKT, D + 1), dtype=bf16)
        vh[..., :D] = vt.astype(bf16)
        vh[..., D] = 1.0
        in_maps.append({"qt": qt, "kt": kt, "vh": vh, "wt": wt, "bb": bb})

    if "nc" not in _NC_CACHE:
        _NC_CACHE["nc"] = build_nc()
    nc = _NC_CACHE["nc"]

    global _LAST_IN_MAPS
    _LAST_IN_MAPS = in_maps

    res = run_bass_kernel_spmd(nc, in_maps, list(range(NCORES)))

    out = np.empty((B * H, S, D), dtype=np.float32)
    for c in range(NCORES):
        o = res.results[c]["out"]  # [PAIRS, 128, NQT*D], q = t*128 + p
        out[c * PAIRS:(c + 1) * PAIRS] = (
            o.reshape(PAIRS, 128, NQT, D).transpose(0, 2, 1, 3).reshape(PAIRS, S, D)
        )
    return out.reshape(B, H, S, D)


# revision 10
# speedup vs baseline: 1.9053x; 1.1964x over previous
"""Multi-head attention + out-projection on 8 TRN2 NeuronCores.

Reference computation (per batch b, head h):
    S = Q K^T / sqrt(64);  P = softmax(S, axis=-1);  O = P V
    OUT = O @ W_out^T + b_out

Sharding: B*H = 64 (b,h) pairs split across 8 cores (8 pairs/core);
attention is fully local per pair, out-proj weights replicated.

Device-side structure (per core):
  - Pairs are processed two at a time (A,B) stacked in SBUF partitions
    0-63 / 64-127. The QK^T matmuls contract over d=64, so A and B
    land on disjoint PE row-groups (tile_position (0,0) vs (64,0)) and
    run concurrently on the systolic array; the per-head S^T PSUM
    tiles also act as each other's double buffer.
  - S^T tiles [128 k, 1024 q] in PSUM; exp on ScalarE with the 1/8
    score scale folded into the activation; no max-subtraction
    (scores are O(+-7), exp stays inside f32/bf16 range).
  - V carries an appended ones-column so the PV matmul produces both
    O^T (partitions 0-63) and the softmax row-sums (partition 64) in
    one accumulation.
  - Normalization: copy O^T to SBUF, DMA-broadcast the rowsum row
    across partitions (stride-0 source AP), reciprocal + multiply on
    VectorE.
  - Out-proj: lhsT = normalized O^T slices, rhs = W_out^T -> natural
    [q, e] layout; the 8 q-tiles of a chunk share one PSUM bank; bias
    added by VectorE in one [128, 512] op per chunk.

Host prep (plain numpy, free): Q/K pre-transposed to [d, s] bf16;
V k-tiled p-major with ones-column, bf16; W_out^T bf16; bias
pre-broadcast/tiled f32.
"""

import numpy as np
import ml_dtypes

from concourse import bass, bacc, tile, mybir
from concourse.bass_utils import run_bass_kernel_spmd

B, H, S, D = 4, 16, 2048, 64
NCORES = 8
PAIRS = (B * H) // NCORES  # 8 (b,h) pairs per core
NKT = S // 128             # 16 key tiles
NQT = S // 128             # 16 query tiles
CHUNK = 1024               # query-column chunk (2 PSUM banks)
NCHUNK = S // CHUNK

_NC_CACHE = {}


def build_nc():
    f32, bf16 = mybir.dt.float32, mybir.dt.bfloat16
    nc = bacc.Bacc(None, target_bir_lowering=False)

    qt_d = nc.declare_dram_parameter("qt", [PAIRS, D, S], bf16, isOutput=False)
    kt_d = nc.declare_dram_parameter("kt", [PAIRS, 128, S], bf16, isOutput=False)
    vh_d = nc.declare_dram_parameter("vh", [PAIRS, 128, NKT, 128], bf16, isOutput=False)
    wt_d = nc.declare_dram_parameter("wt", [D, D], bf16, isOutput=False)
    bb_d = nc.declare_dram_parameter("bb", [128, NQT * D], f32, isOutput=False)
    out_d = nc.declare_dram_parameter("out", [PAIRS, 128, NQT * D], f32, isOutput=True)

    EXPF = mybir.ActivationFunctionType.Exp
    MULT = mybir.AluOpType.mult
    ADD = mybir.AluOpType.add

    with tile.TileContext(nc) as tc:
        with (
            tc.tile_pool(name="const", bufs=1) as constp,
            tc.tile_pool(name="qk", bufs=2) as qkp,
            tc.tile_pool(name="vhp", bufs=2) as vhp,
            tc.tile_pool(name="pt", bufs=8) as ptp,
            tc.tile_pool(name="ep", bufs=2) as epp,
            tc.tile_pool(name="osb", bufs=2) as osbp,
            tc.tile_pool(name="sApsum", bufs=1, space="PSUM") as sAp,
            tc.tile_pool(name="sBpsum", bufs=1, space="PSUM") as sBp,
            tc.tile_pool(name="opsum", bufs=2, space="PSUM") as opsum,
        ):
            wt_sb = constp.tile([D, D], bf16)
            nc.sync.dma_start(wt_sb[:], wt_d[:])
            bb_sb = constp.tile([128, NQT * D], f32)
            nc.sync.dma_start(bb_sb[:], bb_d[:])
            zb = constp.tile([128, 1], f32)
            nc.vector.memset(zb[:], 0.0)

            for pq in range(PAIRS // 2):
                pa, pb = 2 * pq, 2 * pq + 1
                qt2 = qkp.tile([128, S], bf16, tag="qt")
                nc.sync.dma_start(qt2[0:D, :], qt_d[pa])
                nc.sync.dma_start(qt2[D:128, :], qt_d[pb])
                kz2 = [
                    qkp.tile([128, S], bf16, tag="kza", name=f"kza_{pq}"),
                    qkp.tile([128, S], bf16, tag="kzb", name=f"kzb_{pq}"),
                ]
                nc.sync.dma_start(kz2[0][:], kt_d[pa])
                nc.sync.dma_start(kz2[1][:], kt_d[pb])
                vh2 = vhp.tile([128, 2, NKT, 128], bf16)
                nc.sync.dma_start(vh2[:, 0, :, :], vh_d[pa])
                nc.sync.dma_start(vh2[:, 1, :, :], vh_d[pb])
                out2 = [
                    osbp.tile([128, NQT * D], f32, tag="outA", name=f"out_{pq}_A"),
                    osbp.tile([128, NQT * D], f32, tag="outB", name=f"out_{pq}_B"),
                ]
                on2 = [
                    epp.tile([D, S], bf16, tag="onA", name=f"on_{pq}_A"),
                    epp.tile([D, S], bf16, tag="onB", name=f"on_{pq}_B"),
                ]

                for c in range(NCHUNK):
                    q0 = c * CHUNK
                    o_ps = [
                        opsum.tile([128, CHUNK], f32, tag="o", name=f"oA_{pq}_{c}"),
                        opsum.tile([128, CHUNK], f32, tag="o", name=f"oB_{pq}_{c}"),
                    ]
                    for k in range(NKT):
                        s_ps = [
                            sAp.tile([128, CHUNK], f32, tag="s", name=f"sA_{pq}_{c}_{k}"),
                            sBp.tile([128, CHUNK], f32, tag="s", name=f"sB_{pq}_{c}_{k}"),
                        ]
                        # Per head j=0/j=1 share the stationary operand;
                        # alternating heads every 2 MMs keeps row-groups
                        # disjoint so the next head's LDWEIGHTS pulls ahead.
                        for x in (0, 1):
                            for j in (0, 1):
                                nc.tensor.matmul(
                                    s_ps[x][:, j * 512:(j + 1) * 512],
                                    kz2[x][:, k * 128:(k + 1) * 128],
                                    qt2[:, q0 + j * 512:q0 + (j + 1) * 512],
                                    start=True, stop=True,
                                )
                        p_sb = [None, None]
                        for x in (0, 1):
                            p_sb[x] = ptp.tile([128, CHUNK], bf16, tag="p", name=f"p_{pq}_{c}_{k}_{x}")
                            nc.scalar.activation(p_sb[x][:], s_ps[x][:], EXPF, bias=zb[:], scale=0.125)
                        for x in (0, 1):
                            for j in (0, 1):
                                nc.tensor.matmul(
                                    o_ps[x][:, j * 512:(j + 1) * 512],
                                    vh2[:, x, k, :],
                                    p_sb[x][:, j * 512:(j + 1) * 512],
                                    start=(k == 0), stop=(k == NKT - 1),
                                )

                    for x in (0, 1):
                        o_sb = epp.tile([D + 1, CHUNK], f32, tag="osb", name=f"osb_{pq}_{c}_{x}")
                        nc.vector.tensor_copy(o_sb[:], o_ps[x][0:D + 1, :])
                        rs = epp.tile([1, CHUNK], f32, tag="rs", name=f"rs_{pq}_{c}_{x}")
                        nc.sync.dma_start(rs[:], o_sb[D:D + 1, :])
                        rb = epp.tile([D, CHUNK], f32, tag="rb", name=f"rb_{pq}_{c}_{x}")
                        nc.gpsimd.partition_broadcast(rb[:], rs[:])
                        nc.vector.reciprocal_approx_fast(rb[:], rb[:])
                        nc.vector.tensor_tensor(
                            on2[x][:, q0:q0 + CHUNK], o_sb[0:D, :], rb[:], MULT
                        )

                # out-projection + bias once per pair, after both chunks
                for x in (0, 1):
                    opj = opsum.tile([128, NQT * D], f32, tag="o", name=f"opj_{pq}_{x}")
                    for t in range(NQT):
                        nc.tensor.matmul(
                            opj[:, t * D:(t + 1) * D],
                            on2[x][:, t * 128:(t + 1) * 128],
                            wt_sb[:],
                            start=True, stop=True,
                        )
                    nc.vector.tensor_tensor(out2[x][:], opj[:], bb_sb[:], ADD)

                nc.sync.dma_start(out_d[pa], out2[0][:])
                nc.sync.dma_start(out_d[pb], out2[1][:])

    nc.compile()
    return nc


def kernel(queries, keys, values, W_out, b_out):
    bf16 = ml_dtypes.bfloat16

    q = np.asarray(queries, dtype=np.float32).reshape(B * H, S, D)
    k = np.asarray(keys, dtype=np.float32).reshape(B * H, S, D)
    v = np.asarray(values, dtype=np.float32).reshape(B * H, S, D)

    wt = np.ascontiguousarray(np.asarray(W_out, dtype=np.float32).T).astype(bf16)
    bb = np.ascontiguousarray(
        np.tile(np.asarray(b_out, dtype=np.float32), (128, NQT))
    )

    in_maps = []
    for c in range(NCORES):
        sl = slice(c * PAIRS, (c + 1) * PAIRS)
        qt = np.ascontiguousarray(q[sl].transpose(0, 2, 1)).astype(bf16)
        # K^T zero-padded to 128 contraction rows: even pairs occupy rows
        # 0-63, odd pairs rows 64-127 (matching their slot in the stacked
        # qt2 rhs; the zero rows annihilate the other head's queries).
        kt = np.zeros((PAIRS, 128, S), dtype=bf16)
        for pp in range(PAIRS):
            r0 = (pp % 2) * D
            kt[pp, r0:r0 + D] = k[sl][pp].T.astype(bf16)
        # [pairs, S, D] -> k-tiled p-major [pairs, 128, NKT, 128]: cols 0-63
        # V, col 64 ones (softmax denominator), cols 65-127 zero padding.
        vt = v[sl].reshape(PAIRS, NKT, 128, D).transpose(0, 2, 1, 3)
        vh = np.zeros((PAIRS, 128, NKT, 128), dtype=bf16)
        vh[..., :D] = vt.astype(bf16)
        vh[..., D] = 1.0
        in_maps.append({"qt": qt, "kt": kt, "vh": vh, "wt": wt, "bb": bb})

    if "nc" not in _NC_CACHE:
        _NC_CACHE["nc"] = build_nc()
    nc = _NC_CACHE["nc"]

    global _LAST_IN_MAPS
    _LAST_IN_MAPS = in_maps

    res = run_bass_kernel_spmd(nc, in_maps, list(range(NCORES)))

    out = np.empty((B * H, S, D), dtype=np.float32)
    for c in range(NCORES):
        o = res.results[c]["out"]  # [PAIRS, 128, NQT*D], q = t*128 + p
        out[c * PAIRS:(c + 1) * PAIRS] = (
            o.reshape(PAIRS, 128, NQT, D).transpose(0, 2, 1, 3).reshape(PAIRS, S, D)
        )
    return out.reshape(B, H, S, D)
